# revision 6
# baseline (speedup 1.0000x reference)
# Multi-head self-attention (B=2, S=4096, D=512, H=8) on 8 NeuronCores.
#
# Sharding: core c -> batch b = c//4, head-pair hp = c%4 (heads 2hp, 2hp+1,
# i.e. channels [128*hp, 128*hp+128) of the QKV projection space).
# Host pre-slices/transposes weights + x per core (cast fp16 for the PE);
# device does all matmuls; host sums the 4 per-core W_O partials per batch
# (the "all-reduce") and transposes back.
#
# v2 structure (vs the padded-band baseline):
#   qt2/kt2 [128, S]: head0's dk on partitions 0:64, head1's on 64:128 --
#   no zero bands.  The two per-head scores matmuls are K=64 row-tiled
#   (tile_position (0,0) and (64,0)) so they run CONCURRENTLY on the two
#   halves of the PE array: scores cost halves.
#   exp is split across engines: head0 on the ACT engine (true exp),
#   head1 on the DVE as a Schraudolph bit-trick exp (int16(A*s+B) whose
#   bit pattern read as fp16 approximates exp(s/8) to ~+-3%, mean-free;
#   softmax ratio cancels most of it).
#   V is projected directly into [kpos, dk] layout (x chunk stationary,
#   wv moving) -> no PE transposes, no fp32 V staging.
#   W_O outputs evacuate on alternating ACT/DVE, softmax normalization
#   reads the AV psum in place (no staging copy).
# All pools stay open for the whole kernel; PSUM slots are shared between
# phases via tags (8 banks total) so phases overlap with per-slot WAR deps.

import numpy as np

B, S, D, H, DK = 2, 4096, 512, 8, 64
P = 128          # partition tile
NQ = 512         # matmul moving free dim (one fp32 PSUM bank)
QCH = 1024       # q-chunk (2 x NQ) => one [128,1024] exp per kpos-chunk
NKC = S // P     # kpos chunks (32)
NST = S // NQ    # s-tiles of 512 (8)
NDC = D // P     # d chunks (4)
NQC = S // QCH   # q chunks (4)

# Schraudolph fp16 exp: bits = round(A*s + BS) read as fp16 ~= exp(s/8).
# A = 1024*log2(e)/8; BS = 15360 - 1024*c, c = ln(ln2^-1 - 1 ... ) chosen to
# center the multiplicative error of the linear-mantissa approximation:
# c = (ln(1/ln2) - 1 + ln2) / (2 ln 2) = 0.0430357, so error in
# [2^-0.0430, 2^+0.0430] ~= [-2.94%, +3.03%].
EXP_A = 1024.0 * 1.4426950408889634 / 8.0      # 184.66496...
EXP_B = 15360.0 - 1024.0 * 0.04303566

TRACE = False            # test.py sets True to get exec_time_ns + perfetto
TMPDIR = None            # optional trace output dir
LAST_RESULTS = None      # BassKernelResults of the last run (for test.py)

_CACHE = {}


def _build_nc():
    import concourse.bass as bass  # noqa: F401
    import concourse.mybir as mybir
    import concourse.tile as tile
    from concourse import bacc

    f32 = mybir.dt.float32
    f16 = mybir.dt.float16
    i16 = mybir.dt.int16
    Act = mybir.ActivationFunctionType
    Alu = mybir.AluOpType

    nc = bacc.Bacc("TRN2", target_bir_lowering=False, debug=False, num_devices=8)

    xT = nc.dram_tensor("xT", [D, S], f16, kind="ExternalInput")
    wqT = nc.dram_tensor("wqT", [D, P], f16, kind="ExternalInput")
    wkT = nc.dram_tensor("wkT", [D, P], f16, kind="ExternalInput")
    wvT = nc.dram_tensor("wvT", [D, P], f16, kind="ExternalInput")
    woT0 = nc.dram_tensor("woT0", [DK, D], f16, kind="ExternalInput")
    woT1 = nc.dram_tensor("woT1", [DK, D], f16, kind="ExternalInput")
    yT = nc.dram_tensor("yT", [D, S], f32, kind="ExternalOutput")

    with tile.TileContext(nc) as tc:
        with (
            tc.tile_pool(name="sb", bufs=1) as sb,
            tc.tile_pool(name="ps", bufs=1, space="PSUM") as psp,
        ):
            # PSUM budget (8 banks, slots shared across phases by tag):
            #   sc0, sc1: [128,1024] -> 2 banks each (scores / exp staging)
            #   av00..av11: [128,512] -> 1 bank each (AV accum; also used by
            #   the QKV-projection psum tiles and the W_O psum tiles)
            def av_ps(i, shape):
                return psp.tile(shape, f32, tag=f"av{i % 4}", name=f"avps{i}")

            # ---- persistent operand tiles -----------------------------------
            qt2 = sb.tile([P, S], f16, tag="qt2")
            kt2 = sb.tile([P, S], f16, tag="kt2")
            vb = [sb.tile([P, NKC * (DK + 1)], f16, tag=f"vb{h}", name=f"vb{h}")
                  for h in range(2)]
            outtz = [sb.tile([P, S], f16, tag=f"outtz{h}", name=f"outtz{h}")
                     for h in range(2)]
            wosz = [sb.tile([P, D], f16, tag=f"wosz{h}", name=f"wosz{h}")
                    for h in range(2)]

            # ones-fill + zero bands on the (otherwise idle) gpsimd engine.
            # vb is needed by the first AV matmul (ones col -> denominator);
            # outtz/wosz zero bands are needed by phase 3 only.
            nc.gpsimd.memset(vb[0][:, :], 1.0)
            nc.gpsimd.memset(vb[1][:, :], 1.0)
            nc.gpsimd.memset(outtz[0][DK:P, :], 0.0)
            nc.gpsimd.memset(outtz[1][DK:P, :], 0.0)
            nc.gpsimd.memset(wosz[0][DK:P, :], 0.0)
            nc.gpsimd.memset(wosz[1][DK:P, :], 0.0)

            # ---- phase 1: load x + weights, QKV projections -----------------
            xts = [sb.tile([P, S], f16, tag=f"xt{dc}", name=f"xt{dc}")
                   for dc in range(NDC)]
            wsb = {}
            for name, dram in (("q", wqT), ("k", wkT), ("v", wvT)):
                w = sb.tile([P, NDC * P], f16, tag=f"w{name}", name=f"w{name}")
                for dc in range(NDC):
                    nc.sync.dma_start(
                        w[:, dc * P:(dc + 1) * P], dram[dc * P:(dc + 1) * P, :]
                    )
                wsb[name] = w
            nc.sync.dma_start(wosz[0][0:DK, :], woT0[:, :])
            nc.sync.dma_start(wosz[1][0:DK, :], woT1[:, :])
            for blk in range(8):
                sl = slice(blk * NQ, (blk + 1) * NQ)
                for dc in range(NDC):
                    nc.sync.dma_start(xts[dc][:, sl], xT[dc * P:(dc + 1) * P, sl])

            # Q and K projections into [proj-channel, s] layout (this is
            # already Q^T/K^T per head stacked: h0 rows 0:64, h1 rows 64:128).
            # V is projected per 128-s-block into [s, channel] layout
            # directly (x chunk stationary), giving vb's [kpos, dk] chunks
            # without any transposes.
            psn = 0
            for st in range(NST):
                sl = slice(st * NQ, (st + 1) * NQ)
                for name in ("q", "k"):
                    w = wsb[name]
                    ps = av_ps(psn, [P, NQ])
                    psn += 1
                    for dc in range(NDC):
                        nc.tensor.matmul(
                            ps[:, :],
                            w[:, dc * P:(dc + 1) * P],
                            xts[dc][:, sl],
                            start=(dc == 0),
                            stop=(dc == NDC - 1),
                        )
                    if name == "q":
                        nc.vector.tensor_copy(qt2[:, sl], ps[:, :])
                    else:
                        nc.scalar.copy(kt2[:, sl], ps[:, :])
                for ch in range(4 * st, 4 * st + 4):
                    vps = av_ps(psn, [P, P])
                    psn += 1
                    for dc in range(NDC):
                        nc.tensor.matmul(
                            vps[:, :],
                            xts[dc][:, ch * P:(ch + 1) * P],
                            wsb["v"][:, dc * P:(dc + 1) * P],
                            start=(dc == 0),
                            stop=(dc == NDC - 1),
                        )
                    c0 = ch * (DK + 1)
                    nc.scalar.copy(vb[0][:, c0:c0 + DK], vps[:, 0:DK])
                    nc.vector.tensor_copy(vb[1][:, c0:c0 + DK], vps[:, DK:P])

            # ---- phase 2: flash attention -----------------------------------
            def emit_normalize(qc, av):
                # softmax division reading the AV psum directly; the psum
                # slot frees once the tensor_mul has consumed it
                for h in range(2):
                    for sub in range(2):
                        a = av[h, sub]
                        rc = sb.tile([P, NQ], f32, tag="rc", bufs=2)
                        nc.vector.reciprocal_approx_fast(
                            rc[DK:DK + 1, :], a[DK:DK + 1, :]
                        )
                        rcz = sb.tile([P, NQ], f32, tag="rcz", bufs=2)
                        nc.sync.dma_start(rcz[0:1, :], rc[DK:DK + 1, :])
                        rcb = sb.tile([DK, NQ], f32, tag="rcb", bufs=2)
                        nc.gpsimd.partition_broadcast(
                            rcb[:, :], rcz[0:1, :], channels=DK
                        )
                        q0 = qc * QCH + sub * NQ
                        nc.vector.tensor_mul(
                            outtz[h][0:DK, q0:q0 + NQ], a[0:DK, :], rcb[:, :]
                        )

            pending = None
            for qc in range(NQC):
                av = {}
                for h in range(2):
                    for sub in range(2):
                        av[h, sub] = av_ps(psn, [P, NQ])
                        psn += 1
                for k in range(NKC):
                    if k == 3 and pending is not None:
                        emit_normalize(*pending)
                        pending = None
                    # scores^T[kpos, q] for both heads, K=64 row-tiled so the
                    # two heads' matmuls run concurrently on the PE halves
                    scps = [psp.tile([P, QCH], f32, tag=f"sc{h}",
                                     name=f"sc{h}") for h in range(2)]
                    # interleave heads so consecutive matmuls hit different
                    # PE row-groups (rows 0:64 vs 64:128) and run concurrently
                    for sub in range(2):
                        q0 = qc * QCH + sub * NQ
                        for h in range(2):
                            hsl = slice(h * DK, (h + 1) * DK)
                            nc.tensor.matmul(
                                scps[h][:, sub * NQ:(sub + 1) * NQ],
                                kt2[hsl, k * P:(k + 1) * P],
                                qt2[hsl, q0:q0 + NQ],
                                start=True,
                                stop=True,
                            )
                    # exp: head0 on ACT (true exp), head1 on DVE (Schraudolph
                    # int16 bit-trick, read back as fp16 by the AV matmul)
                    ex0 = sb.tile([P, QCH], f16, tag="ex0", name="ex0", bufs=3)
                    nc.scalar.activation(
                        ex0[:, :], scps[0][:, :], Act.Exp, scale=0.125
                    )
                    ex1i = sb.tile([P, QCH], i16, tag="ex1", name="ex1", bufs=3)
                    nc.vector.tensor_scalar(
                        ex1i[:, :], scps[1][:, :], EXP_A, EXP_B,
                        Alu.mult, Alu.add,
                    )
                    exs = [ex0, ex1i.bitcast(f16)]
                    c0 = k * (DK + 1)
                    for h in range(2):
                        for sub in range(2):
                            nc.tensor.matmul(
                                av[h, sub][0:DK + 1, :],
                                vb[h][:, c0:c0 + DK + 1],
                                exs[h][:, sub * NQ:(sub + 1) * NQ],
                                start=(k == 0),
                                stop=(k == NKC - 1),
                            )
                pending = (qc, av)
            emit_normalize(*pending)

            # ---- phase 3: W_O row-slice projection --------------------------
            # yT[e, s] = sum_h wosz_h.T @ outtz_h (K padded to 128 w/ zeros);
            # result DMAs straight from PSUM to DRAM
            for st in range(NST):
                for ec in range(NDC):
                    yp = av_ps(psn, [P, NQ])
                    psn += 1
                    for h in range(2):
                        nc.tensor.matmul(
                            yp[:, :],
                            wosz[h][:, ec * P:(ec + 1) * P],
                            outtz[h][:, st * NQ:(st + 1) * NQ],
                            start=(h == 0),
                            stop=(h == 1),
                        )
                    ys = sb.tile([P, NQ], f32, tag="ys", bufs=4)
                    if ec % 2 == 0:
                        nc.scalar.copy(ys[:, :], yp[:, :])
                    else:
                        nc.vector.tensor_copy(ys[:, :], yp[:, :])
                    nc.sync.dma_start(
                        yT[ec * P:(ec + 1) * P, st * NQ:(st + 1) * NQ],
                        ys[:, :],
                    )

    nc.compile()
    return nc


def kernel(x, wq, wk, wv, wo):
    global LAST_RESULTS
    from concourse.bass_utils import run_bass_kernel_spmd

    if "nc" not in _CACHE:
        _CACHE["nc"] = _build_nc()
    nc = _CACHE["nc"]

    x = np.asarray(x, dtype=np.float32)
    wq = np.asarray(wq, dtype=np.float32)
    wk = np.asarray(wk, dtype=np.float32)
    wv = np.asarray(wv, dtype=np.float32)
    wo = np.asarray(wo, dtype=np.float32)

    in_maps = []
    for c in range(8):
        b, hp = divmod(c, 4)
        e0 = hp * P
        in_maps.append({
            "xT": np.ascontiguousarray(x[b].T.astype(np.float16)),
            "wqT": np.ascontiguousarray(wq[e0:e0 + P].T.astype(np.float16)),
            "wkT": np.ascontiguousarray(wk[e0:e0 + P].T.astype(np.float16)),
            "wvT": np.ascontiguousarray(wv[e0:e0 + P].T.astype(np.float16)),
            "woT0": np.ascontiguousarray(wo[:, e0:e0 + DK].T.astype(np.float16)),
            "woT1": np.ascontiguousarray(wo[:, e0 + DK:e0 + P].T.astype(np.float16)),
        })

    res = run_bass_kernel_spmd(
        nc, in_maps, core_ids=list(range(8)), trace=TRACE, tmpdir=TMPDIR
    )
    LAST_RESULTS = res

    y = np.zeros((B, S, D), dtype=np.float32)
    for c in range(8):
        y[c // 4] += res.results[c]["yT"].T
    return y


# revision 7
# speedup vs baseline: 1.2389x; 1.2389x over previous
# Multi-head self-attention (B=2, S=4096, D=512, H=8) on 8 NeuronCores.
#
# Sharding: core c -> batch b = c//4, head-pair hp = c%4 (heads 2hp, 2hp+1,
# i.e. channels [128*hp, 128*hp+128) of the QKV projection space).
# Host pre-slices/transposes weights + x per core (cast fp16 for the PE);
# device does all matmuls; host sums the 4 per-core W_O partials per batch
# (the "all-reduce") and transposes back.
#
# v2 structure (vs the padded-band baseline):
#   qt2/kt2 [128, S]: head0's dk on partitions 0:64, head1's on 64:128 --
#   no zero bands.  The two per-head scores matmuls are K=64 row-tiled
#   (tile_position (0,0) and (64,0)) so they run CONCURRENTLY on the two
#   halves of the PE array: scores cost halves.
#   exp is split across engines: head0 on the ACT engine (true exp),
#   head1 on the DVE as a Schraudolph bit-trick exp (int16(A*s+B) whose
#   bit pattern read as fp16 approximates exp(s/8) to ~+-3%, mean-free;
#   softmax ratio cancels most of it).
#   V is projected directly into [kpos, dk] layout (x chunk stationary,
#   wv moving) -> no PE transposes, no fp32 V staging.
#   W_O outputs evacuate on alternating ACT/DVE, softmax normalization
#   reads the AV psum in place (no staging copy).
# All pools stay open for the whole kernel; PSUM slots are shared between
# phases via tags (8 banks total) so phases overlap with per-slot WAR deps.

import numpy as np

B, S, D, H, DK = 2, 4096, 512, 8, 64
P = 128          # partition tile
NQ = 512         # matmul moving free dim (one fp32 PSUM bank)
QCH = 1024       # q-chunk (2 x NQ) => one [128,1024] exp per kpos-chunk
NKC = S // P     # kpos chunks (32)
NST = S // NQ    # s-tiles of 512 (8)
NDC = D // P     # d chunks (4)
NQC = S // QCH   # q chunks (4)

# Schraudolph fp16 exp: bits = round(A*s + BS) read as fp16 ~= exp(s/8).
# A = 1024*log2(e)/8; BS = 15360 - 1024*c, c = ln(ln2^-1 - 1 ... ) chosen to
# center the multiplicative error of the linear-mantissa approximation:
# c = (ln(1/ln2) - 1 + ln2) / (2 ln 2) = 0.0430357, so error in
# [2^-0.0430, 2^+0.0430] ~= [-2.94%, +3.03%].
EXP_A = 1024.0 * 1.4426950408889634 / 8.0      # 184.66496...
EXP_B = 15360.0 - 1024.0 * 0.04303566

TRACE = False            # test.py sets True to get exec_time_ns + perfetto
TMPDIR = None            # optional trace output dir
LAST_RESULTS = None      # BassKernelResults of the last run (for test.py)

_CACHE = {}


def _build_nc():
    import concourse.bass as bass  # noqa: F401
    import concourse.mybir as mybir
    import concourse.tile as tile
    from concourse import bacc

    f32 = mybir.dt.float32
    f16 = mybir.dt.float16
    i16 = mybir.dt.int16
    Act = mybir.ActivationFunctionType
    Alu = mybir.AluOpType

    nc = bacc.Bacc("TRN2", target_bir_lowering=False, debug=False, num_devices=8)

    xT = nc.dram_tensor("xT", [D, S], f16, kind="ExternalInput")
    wqT = nc.dram_tensor("wqT", [D, P], f16, kind="ExternalInput")
    wkT = nc.dram_tensor("wkT", [D, P], f16, kind="ExternalInput")
    wvT = nc.dram_tensor("wvT", [D, P], f16, kind="ExternalInput")
    woT0 = nc.dram_tensor("woT0", [DK, D], f16, kind="ExternalInput")
    woT1 = nc.dram_tensor("woT1", [DK, D], f16, kind="ExternalInput")
    yT = nc.dram_tensor("yT", [D, S], f32, kind="ExternalOutput")

    with tile.TileContext(nc) as tc:
        with (
            tc.tile_pool(name="sb", bufs=1) as sb,
            tc.tile_pool(name="ps", bufs=1, space="PSUM") as psp,
        ):
            # PSUM budget (8 banks, slots shared across phases by tag):
            #   sc0, sc1: [128,1024] -> 2 banks each (scores / exp staging)
            #   av00..av11: [128,512] -> 1 bank each (AV accum; also used by
            #   the QKV-projection psum tiles and the W_O psum tiles)
            def av_ps(i, shape):
                return psp.tile(shape, f32, tag=f"av{i % 4}", name=f"avps{i}")

            # ---- persistent operand tiles -----------------------------------
            qt2 = sb.tile([P, S], f16, tag="qt2")
            kt2 = sb.tile([P, S], f16, tag="kt2")
            vb = [sb.tile([P, NKC * (DK + 1)], f16, tag=f"vb{h}", name=f"vb{h}")
                  for h in range(2)]
            outtz = [sb.tile([P, S], f16, tag=f"outtz{h}", name=f"outtz{h}")
                     for h in range(2)]
            wosz = [sb.tile([P, D], f16, tag=f"wosz{h}", name=f"wosz{h}")
                    for h in range(2)]

            # ones-fill + zero bands on the (otherwise idle) gpsimd engine.
            # vb is needed by the first AV matmul (ones col -> denominator);
            # outtz/wosz zero bands are needed by phase 3 only.
            nc.gpsimd.memset(vb[0][:, :], 1.0)
            nc.gpsimd.memset(vb[1][:, :], 1.0)
            nc.gpsimd.memset(outtz[0][DK:P, :], 0.0)
            nc.gpsimd.memset(outtz[1][DK:P, :], 0.0)
            nc.gpsimd.memset(wosz[0][DK:P, :], 0.0)
            nc.gpsimd.memset(wosz[1][DK:P, :], 0.0)

            # ---- phase 1: load x + weights, QKV projections -----------------
            xts = [sb.tile([P, S], f16, tag=f"xt{dc}", name=f"xt{dc}")
                   for dc in range(NDC)]
            wsb = {}
            for name, dram in (("q", wqT), ("k", wkT), ("v", wvT)):
                w = sb.tile([P, NDC * P], f16, tag=f"w{name}", name=f"w{name}")
                for dc in range(NDC):
                    nc.sync.dma_start(
                        w[:, dc * P:(dc + 1) * P], dram[dc * P:(dc + 1) * P, :]
                    )
                wsb[name] = w
            nc.sync.dma_start(wosz[0][0:DK, :], woT0[:, :])
            nc.sync.dma_start(wosz[1][0:DK, :], woT1[:, :])
            for blk in range(8):
                sl = slice(blk * NQ, (blk + 1) * NQ)
                for dc in range(NDC):
                    nc.sync.dma_start(xts[dc][:, sl], xT[dc * P:(dc + 1) * P, sl])

            # Q and K projections into [proj-channel, s] layout (this is
            # already Q^T/K^T per head stacked: h0 rows 0:64, h1 rows 64:128).
            # V is projected per 128-s-block into [s, channel] layout
            # directly (x chunk stationary), giving vb's [kpos, dk] chunks
            # without any transposes.
            psn = 0
            for st in range(NST):
                sl = slice(st * NQ, (st + 1) * NQ)
                for name in ("q", "k"):
                    w = wsb[name]
                    ps = av_ps(psn, [P, NQ])
                    psn += 1
                    for dc in range(NDC):
                        nc.tensor.matmul(
                            ps[:, :],
                            w[:, dc * P:(dc + 1) * P],
                            xts[dc][:, sl],
                            start=(dc == 0),
                            stop=(dc == NDC - 1),
                        )
                    if name == "q":
                        nc.vector.tensor_copy(qt2[:, sl], ps[:, :])
                    else:
                        nc.scalar.copy(kt2[:, sl], ps[:, :])
                for ch in range(4 * st, 4 * st + 4):
                    vps = av_ps(psn, [P, P])
                    psn += 1
                    for dc in range(NDC):
                        nc.tensor.matmul(
                            vps[:, :],
                            xts[dc][:, ch * P:(ch + 1) * P],
                            wsb["v"][:, dc * P:(dc + 1) * P],
                            start=(dc == 0),
                            stop=(dc == NDC - 1),
                        )
                    c0 = ch * (DK + 1)
                    nc.scalar.copy(vb[0][:, c0:c0 + DK], vps[:, 0:DK])
                    nc.vector.tensor_copy(vb[1][:, c0:c0 + DK], vps[:, DK:P])

            # ---- phase 2: flash attention -----------------------------------
            def emit_normalize(qc, av):
                # evacuate av psum fast (frees the bank), then the softmax
                # division off the critical path in SBUF
                for h in range(2):
                    for sub in range(2):
                        a = av[h, sub]
                        raw = sb.tile([DK + 1, NQ], f32, tag=f"raw{h}{sub}",
                                      name=f"raw{h}{sub}", bufs=2)
                        nc.vector.tensor_copy(raw[:, :], a[0:DK + 1, :])
                        dn0 = sb.tile([P, NQ], f32, tag="dn0", bufs=2)
                        nc.sync.dma_start(dn0[0:1, :], raw[DK:DK + 1, :])
                        rc = sb.tile([P, NQ], f32, tag="rc", bufs=2)
                        nc.vector.reciprocal_approx_fast(rc[0:1, :], dn0[0:1, :])
                        rcb = sb.tile([DK, NQ], f32, tag="rcb", bufs=2)
                        nc.gpsimd.partition_broadcast(
                            rcb[:, :], rc[0:1, :], channels=DK
                        )
                        q0 = qc * QCH + sub * NQ
                        nc.vector.tensor_mul(
                            outtz[h][0:DK, q0:q0 + NQ], raw[0:DK, :], rcb[:, :]
                        )

            pending = None
            for qc in range(NQC):
                av = {}
                for h in range(2):
                    for sub in range(2):
                        av[h, sub] = av_ps(psn, [P, NQ])
                        psn += 1
                for k in range(NKC):
                    if k == 3 and pending is not None:
                        emit_normalize(*pending)
                        pending = None
                    # scores^T[kpos, q] for both heads, K=64 row-tiled so the
                    # two heads' matmuls run concurrently on the PE halves
                    scps = [psp.tile([P, QCH], f32, tag=f"sc{h}",
                                     name=f"sc{h}") for h in range(2)]
                    # interleave heads so consecutive matmuls hit different
                    # PE row-groups (rows 0:64 vs 64:128) and run concurrently
                    for sub in range(2):
                        q0 = qc * QCH + sub * NQ
                        for h in range(2):
                            hsl = slice(h * DK, (h + 1) * DK)
                            nc.tensor.matmul(
                                scps[h][:, sub * NQ:(sub + 1) * NQ],
                                kt2[hsl, k * P:(k + 1) * P],
                                qt2[hsl, q0:q0 + NQ],
                                start=True,
                                stop=True,
                            )
                    # exp: head0 on ACT (true exp), head1 on DVE (Schraudolph
                    # int16 bit-trick, read back as fp16 by the AV matmul)
                    ex0 = sb.tile([P, QCH], f16, tag="ex0", name="ex0", bufs=3)
                    nc.scalar.activation(
                        ex0[:, :], scps[0][:, :], Act.Exp, scale=0.125
                    )
                    ex1i = sb.tile([P, QCH], i16, tag="ex1", name="ex1", bufs=3)
                    nc.vector.tensor_scalar(
                        ex1i[:, :], scps[1][:, :], EXP_A, EXP_B,
                        Alu.mult, Alu.add,
                    )
                    exs = [ex0, ex1i.bitcast(f16)]
                    c0 = k * (DK + 1)
                    for h in range(2):
                        for sub in range(2):
                            nc.tensor.matmul(
                                av[h, sub][0:DK + 1, :],
                                vb[h][:, c0:c0 + DK + 1],
                                exs[h][:, sub * NQ:(sub + 1) * NQ],
                                start=(k == 0),
                                stop=(k == NKC - 1),
                            )
                pending = (qc, av)
            emit_normalize(*pending)

            # ---- phase 3: W_O row-slice projection --------------------------
            # yT[e, s] = sum_h wosz_h.T @ outtz_h (K padded to 128 w/ zeros);
            # result DMAs straight from PSUM to DRAM
            for st in range(NST):
                for ec in range(NDC):
                    yp = av_ps(psn, [P, NQ])
                    psn += 1
                    for h in range(2):
                        nc.tensor.matmul(
                            yp[:, :],
                            wosz[h][:, ec * P:(ec + 1) * P],
                            outtz[h][:, st * NQ:(st + 1) * NQ],
                            start=(h == 0),
                            stop=(h == 1),
                        )
                    ys = sb.tile([P, NQ], f32, tag="ys", bufs=4)
                    if ec % 2 == 0:
                        nc.scalar.copy(ys[:, :], yp[:, :])
                    else:
                        nc.vector.tensor_copy(ys[:, :], yp[:, :])
                    nc.sync.dma_start(
                        yT[ec * P:(ec + 1) * P, st * NQ:(st + 1) * NQ],
                        ys[:, :],
                    )

    nc.compile()
    return nc


def kernel(x, wq, wk, wv, wo):
    global LAST_RESULTS
    from concourse.bass_utils import run_bass_kernel_spmd

    if "nc" not in _CACHE:
        _CACHE["nc"] = _build_nc()
    nc = _CACHE["nc"]

    x = np.asarray(x, dtype=np.float32)
    wq = np.asarray(wq, dtype=np.float32)
    wk = np.asarray(wk, dtype=np.float32)
    wv = np.asarray(wv, dtype=np.float32)
    wo = np.asarray(wo, dtype=np.float32)

    in_maps = []
    for c in range(8):
        b, hp = divmod(c, 4)
        e0 = hp * P
        in_maps.append({
            "xT": np.ascontiguousarray(x[b].T.astype(np.float16)),
            "wqT": np.ascontiguousarray(wq[e0:e0 + P].T.astype(np.float16)),
            "wkT": np.ascontiguousarray(wk[e0:e0 + P].T.astype(np.float16)),
            "wvT": np.ascontiguousarray(wv[e0:e0 + P].T.astype(np.float16)),
            "woT0": np.ascontiguousarray(wo[:, e0:e0 + DK].T.astype(np.float16)),
            "woT1": np.ascontiguousarray(wo[:, e0 + DK:e0 + P].T.astype(np.float16)),
        })

    res = run_bass_kernel_spmd(
        nc, in_maps, core_ids=list(range(8)), trace=TRACE, tmpdir=TMPDIR
    )
    LAST_RESULTS = res

    y = np.zeros((B, S, D), dtype=np.float32)
    for c in range(8):
        y[c // 4] += res.results[c]["yT"].T
    return y
